# revision 27
# baseline (speedup 1.0000x reference)
"""3-layer GAT (GATConv x3, PyG-style) on 8 Trainium2 NeuronCores.

Strategy (dst-sharding / edge parallelism):
  - Host picks a node permutation Pi: nodes are dealt round-robin (by in-degree
    rank) to the 8 cores, so every core owns 6250 destination nodes with a
    near-identical degree profile, sorted by degree desc within the core.
    All device-side node data lives in Pi order; host un-permutes at the end.
  - Per layer, each core computes "table" rows [h | alpha_src | alpha_dst] for
    its own nodes with one PE matmul per 128-node tile (rhs is W augmented with
    W@a_src / W@a_dst columns), then the 8 shards are AllGather-ed so every
    core holds the full node table for gathers.
  - Edge phase: edges grouped by destination (ELL layout, 128 dsts per tile,
    per-tile max-degree K from a degree-sorted schedule shared by all cores).
    Source rows are fetched with indirect (gathering) DMA using int32 indices.
    Padding slots point to a sentinel row with alpha_src = -1e30 so their
    softmax weight is exactly 0.
  - Softmax without max-subtraction (scores are O(10), fp32-safe):
    w = exp(leaky_relu(a_s[src]+a_d[dst])); denom accumulated per dst;
    weighted message sum via identity-lhsT matmuls accumulating in PSUM
    (rhs pre-scaled by w with one broadcast tensor_tensor multiply).
  - ELU + transpose feeds the next layer's table build (lhsT).

Host/transport optimizations (v2):
  - One persistent jax.jit(shard_map(...)) executable reused across calls
    (the stock run_bass_kernel_spmd path rebuilds + retraces per call).
  - Params / ELL indices are hashed and cached on device; per call only the
    node features ship (as bf16, half the bytes) and the output returns bf16.
  - Donated output buffers are created on device (jnp.zeros) instead of
    shipping host zeros; dispatch is fully async with a single end sync.
  - The edge schedule is memoized on a digest of edge_index.

Device-side precision (v3): weights/activations/table all run bf16 into
fp32 PSUM accumulation (table-build matmuls, gathered rows, attention
weights, message matmuls); softmax scores, denominators and the final
normalize/bias/ELU stay fp32. Measured end-to-end error ~0.7% of output
scale vs the fp32 reference (tolerance 2%). The node table in bf16 also
halves the indirect-gather and AllGather HBM traffic, the dominant
device-side cost (~850k x 528 B random row reads per layer across cores).

GAT_REPS=N (diagnostic) builds the NEFF with the whole 3-layer GAT body
repeated N times: (T(N) - T(1)) / (N-1) isolates pure device time from the
~70 ms axon dispatch round-trip. Measured: ~4.5 ms per iteration.
"""

import hashlib
import os
import sys
import time

import numpy as np

sys.path.insert(0, "/opt/trn_rl_repo")

# ---------------------------------------------------------------- constants
ABL_SKIP_AG = os.environ.get("GAT_SKIP_AG") == "1"
ABL_AG_NONE = os.environ.get("GAT_AG_NONE") == "1"
WIDE_GATHER = os.environ.get("GAT_WIDE_GATHER", "0") == "1"
ABL_SKIP_GATHER = os.environ.get("GAT_SKIP_GATHER") == "1"
TAB_F32 = os.environ.get("GAT_TAB_F32") == "1"
REPS = int(os.environ.get("GAT_REPS", "1"))
NCORES = 8
KCH = 48          # k-slots per gather chunk
PADVAL = -1.0e30  # alpha_src sentinel for padding slots

_RUNNERS = {}


def _layer_cfgs(F_in, H, C, C_out):
    HC = H * C
    return [
        dict(F=F_in, HC=HC, H=H, C=C),
        dict(F=HC, HC=HC, H=H, C=C),
        dict(F=HC, HC=C_out, H=1, C=C_out),
    ]


def _to_bf16(x):
    import ml_dtypes
    x = np.ascontiguousarray(x, dtype=np.float32)
    u = x.view(np.uint32)
    r = ((u + np.uint32(0x7FFF) + ((u >> np.uint32(16)) & np.uint32(1)))
         >> np.uint32(16)).astype(np.uint16)
    return r.view(ml_dtypes.bfloat16)


def _digest(*arrs):
    h = hashlib.blake2b(digest_size=16)
    for a in arrs:
        h.update(np.ascontiguousarray(a))
    return h.hexdigest()


# ================================================================ host prep
def _schedule(edge_index, N):
    """Build the permutation, ELL schedule and per-core index arrays."""
    src = np.asarray(edge_index[0], dtype=np.int64)
    dst = np.asarray(edge_index[1], dtype=np.int64)
    # self-loops are NOT in the ELL lists: the device adds each node's own
    # row as slot 0 of its tile via one direct DMA from the local shard.
    deg = np.bincount(dst, minlength=N) + 1  # +self, for the same ordering
    order = np.argsort(-deg, kind="stable")

    NLOC = N // NCORES
    assert NLOC * NCORES == N
    NT = (NLOC + 127) // 128
    NPADL = NT * 128

    Pi = np.empty(N, dtype=np.int64)
    for c in range(NCORES):
        Pi[c * NLOC:(c + 1) * NLOC] = order[c::NCORES]
    pos = np.empty(N, dtype=np.int64)
    pos[Pi] = np.arange(N, dtype=np.int64)

    psrc = pos[src]
    pdst = pos[dst]
    eo = np.argsort(pdst, kind="stable")
    s_sorted = psrc[eo].astype(np.int32)
    d_sorted = pdst[eo]
    starts = np.searchsorted(d_sorted, np.arange(N + 1))
    degp = (starts[1:] - starts[:-1]).astype(np.int64)  # in-degree by position

    # common per-tile K schedule (max over cores)
    K = np.zeros(NT, dtype=np.int64)
    for t in range(NT):
        lo, hi = t * 128, min((t + 1) * 128, NLOC)
        for c in range(NCORES):
            base = c * NLOC
            if hi > lo:
                K[t] = max(K[t], int(degp[base + lo:base + hi].max()))
        K[t] = max(K[t], 1)
        K[t] += 1 - (K[t] & 1)  # odd K: K+1 slots (with self) stays even
    TOTC = int(K.sum())
    col0 = np.concatenate([[0], np.cumsum(K)]).astype(np.int64)

    # The gathered table is the concat of per-core shards of NLOC+1 rows
    # (row NLOC of every shard is the -1e30 sentinel): remap position p to
    # (p // NLOC) * (NLOC + 1) + p % NLOC; padding slots point at row NLOC.
    s_remap = ((s_sorted // NLOC) * (NLOC + 1) + s_sorted % NLOC).astype(np.int32)
    idxs = np.full((NCORES, 128, TOTC), NLOC, dtype=np.int32)
    for c in range(NCORES):
        for t in range(NT):
            lo = t * 128
            hi = min(lo + 128, NLOC)
            c0 = col0[t]
            for p in range(hi - lo):
                gp = c * NLOC + lo + p
                s0, s1 = starts[gp], starts[gp + 1]
                idxs[c, p, c0:c0 + (s1 - s0)] = s_remap[s0:s1]

    return dict(N=N, NLOC=NLOC, NT=NT, NPADL=NPADL, K=[int(k) for k in K],
                TOTC=TOTC, col0=[int(c) for c in col0], Pi=Pi, idxs=idxs)


# ================================================================ device build
def _build_module(sched, cfgs):
    import concourse.bass as bass
    import concourse.mybir as mybir
    import concourse.tile as tile
    from concourse import bacc
    from concourse.masks import make_identity

    f32 = mybir.dt.float32
    bf16 = mybir.dt.bfloat16
    i32 = mybir.dt.int32
    tdt = f32 if TAB_F32 else bf16

    N = sched["N"]
    NT = sched["NT"]
    NPADL = sched["NPADL"]
    TOTC = sched["TOTC"]
    K = sched["K"]
    col0 = sched["col0"]

    nc = bacc.Bacc("TRN2", target_bir_lowering=False, debug=False,
                   num_devices=NCORES)

    F0 = cfgs[0]["F"]
    assert F0 <= 128
    xT_in = nc.dram_tensor("xT", [F0, NPADL], bf16, kind="ExternalInput")
    idx_in = nc.dram_tensor("idx", [128, TOTC], mybir.dt.uint16,
                            kind="ExternalInput")
    w_in, a_in, b_in = [], [], []
    for l, cfg in enumerate(cfgs):
        F, HC, H, C = cfg["F"], cfg["HC"], cfg["H"], cfg["C"]
        w_in.append(nc.dram_tensor(f"w{l}", [F, HC], bf16, kind="ExternalInput"))
        a_in.append(nc.dram_tensor(f"a{l}", [2 * H, C], bf16, kind="ExternalInput"))
        b_in.append(nc.dram_tensor(f"b{l}", [1, HC], f32, kind="ExternalInput"))
    out_ext = nc.dram_tensor("out", [NPADL, cfgs[-1]["HC"]], bf16,
                             kind="ExternalOutput")

    with tile.TileContext(nc) as tc:
        import contextlib
        with contextlib.ExitStack() as ctx:
            const = ctx.enter_context(tc.tile_pool(name="const", bufs=1))
            sb = ctx.enter_context(tc.tile_pool(name="sb", bufs=2))
            ps = ctx.enter_context(tc.tile_pool(name="ps", bufs=2, space="PSUM"))
            dram = ctx.enter_context(tc.tile_pool(name="dram", bufs=1, space="DRAM"))

            ident = const.tile([128, 128], f32, name="ident")
            make_identity(nc, ident[:])
            ident_bf = const.tile([128, 128], bf16, name="ident_bf")
            make_identity(nc, ident_bf[:])

            idx16 = const.tile([128, TOTC], mybir.dt.uint16, name="idx16")
            nc.sync.dma_start(out=idx16[:], in_=idx_in.ap()[:, :])
            idx_sb = const.tile([128, TOTC], i32, name="idx_sb")
            nc.vector.tensor_copy(out=idx_sb[:], in_=idx16[:])

            # ---------------- pass 1: params in SBUF (once, reused by reps)
            layer_params = []
            for l, cfg in enumerate(cfgs):
                F, HC, H, C = cfg["F"], cfg["HC"], cfg["H"], cfg["C"]
                RL = HC + 2 * H
                FP = (F + 127) // 128
                f_sz = [min(128, F - i * 128) for i in range(FP)]
                HCP = (HC + 127) // 128
                hc_sz = [min(128, HC - i * 128) for i in range(HCP)]
                B_sb = const.tile([128, HC], f32, name=f"B_sb{l}")
                nc.sync.dma_start(
                    out=B_sb[:],
                    in_=b_in[l].ap()[0:1, :].to_broadcast([128, HC]))
                # W rows straight into R; W^T derived on device by PE transpose
                R_sb = []
                for m in range(FP):
                    Rm = const.tile([f_sz[m], RL], bf16, name=f"R_sb{l}_{m}")
                    nc.sync.dma_start(
                        out=Rm[:, 0:HC],
                        in_=w_in[l].ap()[m * 128:m * 128 + f_sz[m], :])
                    R_sb.append(Rm)
                wT_sb = []
                for k in range(HCP):
                    wTk = const.tile([hc_sz[k], F], bf16, name=f"wT_sb{l}_{k}")
                    wT_sb.append(wTk)
                for m in range(FP):
                    for k in range(HCP):
                        wtp = ps.tile([128, 512], bf16, name="wtp", tag="tp",
                                      bufs=2)
                        nc.tensor.transpose(
                            out=wtp[:hc_sz[k], 0:f_sz[m]],
                            in_=R_sb[m][:, k * 128:k * 128 + hc_sz[k]],
                            identity=ident_bf[:])
                        nc.vector.tensor_copy(
                            out=wT_sb[k][:, m * 128:m * 128 + f_sz[m]],
                            in_=wtp[:hc_sz[k], 0:f_sz[m]])

                # block-diagonal a matrix: [HC, 2H]; col j holds a row j
                # (j < H: a_src head j ; j >= H: a_dst head j-H)
                abd = []
                for k in range(HCP):
                    ab = const.tile([hc_sz[k], 2 * H], bf16, name=f"abd{l}_{k}")
                    nc.gpsimd.memset(ab[:], 0.0)
                    abd.append(ab)
                for j in range(2 * H):
                    h = j if j < H else j - H
                    gp0 = h * C
                    kp, lo = gp0 // 128, gp0 % 128
                    nc.sync.dma_start(
                        out=abd[kp][lo:lo + C, j:j + 1],
                        in_=a_in[l].ap()[j, :, None])

                # R = [W | u_src | u_dst]: append u columns
                for m in range(FP):
                    u_ps = ps.tile([f_sz[m], 2 * H], f32, name="u_ps",
                                   tag="Tps", bufs=1)
                    for k in range(HCP):
                        nc.tensor.matmul(
                            out=u_ps[:],
                            lhsT=wT_sb[k][:, m * 128:m * 128 + f_sz[m]],
                            rhs=abd[k][:],
                            start=(k == 0), stop=(k == HCP - 1))
                    nc.vector.tensor_copy(out=R_sb[m][:, HC:HC + 2 * H],
                                          in_=u_ps[:])
                layer_params.append(dict(B_sb=B_sb, R_sb=R_sb))

            # ---------------- pass 2: the 3-layer GAT, repeated REPS times
            # (REPS>1 is a timing diagnostic: per-rep marginal cost is the
            # pure NEFF execution time, free of dispatch overhead)
            for rep in range(REPS):
              xT_src = xT_in.ap()
              for l, cfg in enumerate(cfgs):
                F, HC, H, C = cfg["F"], cfg["HC"], cfg["H"], cfg["C"]
                RL = HC + 2 * H
                FP = (F + 127) // 128
                f_sz = [min(128, F - i * 128) for i in range(FP)]
                HCP = (HC + 127) // 128
                hc_sz = [min(128, HC - i * 128) for i in range(HCP)]
                last = l == len(cfgs) - 1
                B_sb = layer_params[l]["B_sb"]
                R_sb = layer_params[l]["R_sb"]

                # ---------------- build local table shard
                NLOC = sched["NLOC"]
                TSR = max(NPADL, NLOC + 1)
                T_shard = dram.tile([TSR, RL], tdt, name=f"T_shard{l}",
                                    tag=f"T_shard{l}")
                AD_sb = const.tile([128, NT * H], tdt, name=f"AD_sb{l}")
                bgroups = []
                _t = 0
                while _t < NT:
                    if _t + 1 < NT:
                        bgroups.append([_t, _t + 1])
                        _t += 2
                    else:
                        bgroups.append([_t])
                        _t += 1
                for bts in bgroups:
                    ng = len(bts)
                    t0 = bts[0]
                    Tps = ps.tile([128, 1024], f32, name="Tps", tag="Tps",
                                  bufs=1)
                    m = 128 * ng
                    xk = sb.tile([128, FP * 128 * ng], bf16, name="xk",
                                 tag="xk", bufs=3)
                    if FP == 1:
                        nc.sync.dma_start(
                            out=xk[:f_sz[0], 0:m],
                            in_=xT_src[0:f_sz[0], t0 * 128:t0 * 128 + m])
                    else:
                        nc.sync.dma_start(
                            out=xk[:].rearrange("f (a n) -> f a n", a=FP),
                            in_=xT_src[:, t0 * 128:t0 * 128 + m].rearrange(
                                "(a f) n -> f a n", a=FP))
                    for tj in range(ng):
                        for k in range(FP):
                            nc.tensor.matmul(
                                out=Tps[:, tj * 512:tj * 512 + RL],
                                lhsT=xk[:f_sz[k], k * m + tj * 128:
                                        k * m + tj * 128 + 128],
                                rhs=R_sb[k][:],
                                start=(k == 0), stop=(k == FP - 1))
                    Trow = sb.tile([128, ng * RL], tdt, name="Trow",
                                   tag="Trow", bufs=2)
                    nc.vector.tensor_copy(
                        out=Trow[:].rearrange("p (t r) -> p t r", t=ng),
                        in_=Tps[:].rearrange("p (t x) -> p t x", t=ng)
                        [:, :, 0:RL])
                    nc.vector.tensor_copy(
                        out=AD_sb[:, t0 * H:(t0 + ng) * H]
                        .rearrange("p (t h) -> p t h", t=ng),
                        in_=Tps[:].rearrange("p (t x) -> p t x", t=ng)
                        [:, :, HC + H:HC + 2 * H])
                    nc.sync.dma_start(
                        out=T_shard[t0 * 128:(t0 + ng) * 128, :]
                        .rearrange("(t p) r -> p t r", t=ng),
                        in_=Trow[:].rearrange("p (t r) -> p t r", t=ng))

                # ---------------- sentinel row, then all-gather the table
                padrow = const.tile([1, RL], tdt, name=f"padrow{l}")
                nc.gpsimd.memset(padrow[:], 0.0)
                nc.gpsimd.memset(padrow[:, HC:HC + H], PADVAL)
                nc.sync.dma_start(out=T_shard[NLOC:NLOC + 1, :], in_=padrow[:])
                NG = NCORES * (NLOC + 1)
                T_full = dram.tile([NG, RL], tdt, name=f"T_full{l}",
                                   tag=f"T_full{l}", addr_space="Shared")
                if ABL_AG_NONE:
                    pass  # timing ablation: leave T_full stale
                else:
                    nc.gpsimd.collective_compute(
                        "AllGather", mybir.AluOpType.bypass,
                        replica_groups=[list(range(NCORES))],
                        ins=[T_shard[0:NLOC + 1, :]],
                        outs=[T_full[0:NG, :]])

                # ---------------- next-layer x^T target
                if not last:
                    xT_next = dram.tile([HC, NPADL], bf16, name=f"xT_next{l}",
                                        tag=f"xT_next{l}")

                # ---------------- edge phase: tiles in pairs, one
                # eviction op-set per pair (PSUM accumulator [128, 1024])
                PAIR = 512 // HC
                groups = []
                _t = 0
                while _t < NT:
                    if _t + 1 < NT:
                        groups.append([_t, _t + 1])
                        _t += 2
                    else:
                        groups.append([_t])
                        _t += 1
                for ts in groups:
                    ng = len(ts)
                    agg = ps.tile([128, ng * 512], f32, name="agg", tag="agg",
                                  bufs=2)
                    den = sb.tile([128, ng * H], f32, name="den", tag="den",
                                  bufs=2)
                    for tj, t in enumerate(ts):
                        Kt = K[t]
                        kc = Kt + 1  # slot 0: own row (direct); 1..Kt: ELL
                        assert kc <= KCH
                        cbase = col0[t]
                        G = sb.tile([128, KCH * RL], tdt, name="G",
                                    tag="G", bufs=2)
                        G3 = G[:, 0:kc * RL].rearrange(
                            "p (k r) -> p k r", r=RL)
                        # self-loop row is local: one direct (HWDGE) DMA
                        nc.sync.dma_start(
                            out=G[:, 0:RL],
                            in_=T_shard[t * 128:(t + 1) * 128, :])
                        if ABL_SKIP_GATHER:
                            nc.gpsimd.memset(G[:, RL:kc * RL], 1.0)
                        else:
                            for k in range(Kt):
                                nc.gpsimd.indirect_dma_start(
                                    out=G[:, (k + 1) * RL:(k + 2) * RL],
                                    out_offset=None,
                                    in_=T_full[:, :],
                                    in_offset=bass.IndirectOffsetOnAxis(
                                        ap=idx_sb[:,
                                                  cbase + k:cbase + k + 1],
                                        axis=0))
                        # e = alpha_src[src] + alpha_dst[dst]; [p, H, kc]
                        ew = sb.tile([128, H * KCH], f32, name="ew",
                                     tag="ew", bufs=2)
                        e_hk = ew[:, 0:H * kc].rearrange(
                            "p (h k) -> p h k", k=kc)
                        gas = G3[:, :, HC:HC + H].transpose([0, 2, 1])
                        adt = AD_sb[:, t * H:(t + 1) * H][:, :, None] \
                            .to_broadcast([128, H, kc])
                        nc.vector.tensor_tensor(out=e_hk, in0=gas, in1=adt,
                                                op=mybir.AluOpType.add)
                        # leaky_relu(e, 0.2) = max(e, 0.2e)
                        nc.vector.scalar_tensor_tensor(
                            out=ew[:, 0:H * kc], in0=ew[:, 0:H * kc],
                            scalar=0.2, in1=ew[:, 0:H * kc],
                            op0=mybir.AluOpType.mult,
                            op1=mybir.AluOpType.max)
                        ewb = sb.tile([128, H * KCH], tdt, name="ewb",
                                      tag="ewb", bufs=2)
                        nc.scalar.activation(
                            ewb[:, 0:H * kc], ew[:, 0:H * kc],
                            mybir.ActivationFunctionType.Exp)
                        eb_hk = ewb[:, 0:H * kc].rearrange(
                            "p (h k) -> p h k", k=kc)
                        dslice = den[:, tj * H:(tj + 1) * H]
                        nc.vector.tensor_reduce(
                            out=dslice, in_=eb_hk,
                            axis=mybir.AxisListType.X,
                            op=mybir.AluOpType.add)
                        # msg = h[src] * w  (broadcast w over C)
                        msg = sb.tile([128, KCH * HC], tdt, name="msg",
                                      tag="msg", bufs=2)
                        m4 = msg[:, 0:kc * HC].rearrange(
                            "p (k h c) -> p k h c", h=H, c=C)
                        gh = G3[:, :, 0:HC].rearrange(
                            "p k (h c) -> p k h c", c=C)
                        wb = eb_hk.transpose([0, 2, 1])[:, :, :, None] \
                            .to_broadcast([128, kc, H, C])
                        nc.vector.tensor_tensor(out=m4, in0=gh, in1=wb,
                                                op=mybir.AluOpType.mult)
                        npair = (kc + PAIR - 1) // PAIR
                        kcp = npair * PAIR
                        if kcp > kc:
                            nc.gpsimd.memset(msg[:, kc * HC:kcp * HC], 0.0)
                        for k2 in range(npair):
                            w0 = k2 * PAIR
                            nc.tensor.matmul(
                                out=agg[:, tj * 512:(tj + 1) * 512],
                                lhsT=(ident if TAB_F32 else ident_bf)[:],
                                rhs=msg[:, w0 * HC:(w0 + PAIR) * HC],
                                start=(k2 == 0),
                                stop=(k2 == npair - 1))
                    # ---- batched eviction for the group
                    if ts[-1] == NT - 1:
                        # dummy rows (zero denominators) exist only here
                        nc.vector.tensor_scalar_max(den[:], den[:], 1e-30)
                    rcp = sb.tile([128, ng * H], f32, name="rcp", tag="rcp",
                                  bufs=2)
                    nc.vector.reciprocal(rcp[:], den[:])
                    osum = sb.tile([128, ng * HC], f32, name="osum",
                                   tag="osum", bufs=2)
                    nc.vector.tensor_reduce(
                        out=osum[:].rearrange("p (t x) -> p t x", t=ng),
                        in_=agg[:].rearrange("p (t g x) -> p t x g",
                                             t=ng, g=PAIR),
                        axis=mybir.AxisListType.X,
                        op=mybir.AluOpType.add)
                    o = sb.tile([128, ng * HC], f32, name="o", tag="o", bufs=2)
                    nc.vector.tensor_tensor(
                        out=o[:].rearrange("p (t h c) -> p t h c", t=ng, c=C),
                        in0=osum[:].rearrange("p (t h c) -> p t h c",
                                              t=ng, c=C),
                        in1=rcp[:].rearrange("p (t h) -> p t h",
                                             t=ng)[:, :, :, None]
                        .to_broadcast([128, ng, H, C]),
                        op=mybir.AluOpType.mult)
                    nc.vector.tensor_tensor(
                        out=o[:].rearrange("p (t x) -> p t x", t=ng),
                        in0=o[:].rearrange("p (t x) -> p t x", t=ng),
                        in1=B_sb[:][:, None, :].to_broadcast([128, ng, HC]),
                        op=mybir.AluOpType.add)
                    if last:
                        obf = sb.tile([128, ng * HC], bf16, name="obf",
                                      tag="obf", bufs=2)
                        nc.vector.tensor_copy(out=obf[:], in_=o[:])
                        nc.sync.dma_start(
                            out=out_ext.ap()[ts[0] * 128:
                                             (ts[-1] + 1) * 128, :]
                            .rearrange("(t p) c -> p t c", t=ng),
                            in_=obf[:].rearrange("p (t c) -> p t c", t=ng))
                    else:
                        # ELU = relu(x) + min(exp(x)-1, 0)
                        ex = sb.tile([128, ng * HC], f32, name="ex", tag="ex",
                                     bufs=2)
                        nc.scalar.activation(ex[:], o[:],
                                             mybir.ActivationFunctionType.Exp)
                        nc.vector.tensor_scalar(ex[:], ex[:], -1.0, 0.0,
                                                mybir.AluOpType.add,
                                                mybir.AluOpType.min)
                        nc.vector.scalar_tensor_tensor(
                            out=o[:], in0=o[:], scalar=0.0, in1=ex[:],
                            op0=mybir.AluOpType.max,
                            op1=mybir.AluOpType.add)
                        tp = ps.tile([128, ng * HC], f32, name="tp", tag="tp",
                                     bufs=2)
                        for tj in range(ng):
                            for j in range(HCP):
                                nc.tensor.transpose(
                                    out=tp[:hc_sz[j],
                                           (j * ng + tj) * 128:
                                           (j * ng + tj) * 128 + 128],
                                    in_=o[:, tj * HC + j * 128:
                                          tj * HC + j * 128 + hc_sz[j]],
                                    identity=ident[:])
                        tcp = sb.tile([128, ng * HC], bf16, name="tcp",
                                      tag="tcp", bufs=2)
                        nc.vector.tensor_copy(out=tcp[:], in_=tp[:])
                        nc.sync.dma_start(
                            out=xT_next[:, ts[0] * 128:(ts[-1] + 1) * 128]
                            .rearrange("(a f) m -> f a m", a=HCP),
                            in_=tcp[:].rearrange("f (a m) -> f a m", a=HCP))
                if not last:
                    xT_src = xT_next

    nc.compile()
    return nc


# ================================================================ runner
class _Runner:
    """Owns the compiled module, a persistent jitted executable and the
    device-resident static inputs (ELL indices + params)."""

    def __init__(self, sched, cfgs, ekey):
        self.sched = sched
        self.cfgs = cfgs
        self.ekey = ekey
        self.pkey = None
        self.static_dev = {}
        self._last_xT = None
        self.nc = _build_module(sched, cfgs)
        self._setup_jit()

    def _setup_jit(self):
        import jax
        import jax.numpy as jnp
        import concourse.mybir as mybir
        from concourse.bass2jax import (_bass_exec_p, install_neuronx_cc_hook,
                                        partition_id_tensor)
        from jax.sharding import Mesh, NamedSharding, PartitionSpec
        from jax.experimental.shard_map import shard_map

        self.jax = jax
        install_neuronx_cc_hook()
        nc = self.nc
        partition_name = (nc.partition_id_tensor.name
                          if nc.partition_id_tensor else None)
        in_names, out_names, out_avals = [], [], []
        for alloc in nc.m.functions[0].allocations:
            if not isinstance(alloc, mybir.MemoryLocationSet):
                continue
            name = alloc.memorylocations[0].name
            if alloc.kind == "ExternalInput":
                if name != partition_name:
                    in_names.append(name)
            elif alloc.kind == "ExternalOutput":
                out_avals.append(jax.core.ShapedArray(
                    tuple(alloc.tensor_shape), mybir.dt.np(alloc.dtype)))
                out_names.append(name)
        self.in_names = in_names
        self.out_names = out_names
        self.out_avals = out_avals
        n_params = len(in_names)
        n_outs = len(out_avals)
        in_names_all = in_names + out_names
        if partition_name is not None:
            in_names_all.append(partition_name)
        donate = tuple(range(n_params, n_params + n_outs))

        def _body(*args):
            operands = list(args)
            if partition_name is not None:
                operands.append(partition_id_tensor())
            outs = _bass_exec_p.bind(
                *operands, out_avals=tuple(out_avals),
                in_names=tuple(in_names_all), out_names=tuple(out_names),
                lowering_input_output_aliases=(), sim_require_finite=True,
                sim_require_nnan=True, nc=nc)
            return tuple(outs)

        devices = jax.devices()[:NCORES]
        assert len(devices) == NCORES, \
            f"need {NCORES} devices, have {len(jax.devices())}"
        mesh = Mesh(np.asarray(devices), ("core",))
        self.mesh = mesh
        self.core_sharding = NamedSharding(mesh, PartitionSpec("core"))
        in_specs = (PartitionSpec("core"),) * (n_params + n_outs)
        out_specs = (PartitionSpec("core"),) * n_outs
        self.sharded = jax.jit(
            shard_map(_body, mesh=mesh, in_specs=in_specs,
                      out_specs=out_specs, check_rep=False),
            donate_argnums=donate, keep_unused=True)

        zshapes = [(NCORES * a.shape[0], *a.shape[1:]) for a in out_avals]
        zdtypes = [a.dtype for a in out_avals]
        self.make_zeros = jax.jit(
            lambda: tuple(jnp.zeros(s, d) for s, d in zip(zshapes, zdtypes)),
            out_shardings=tuple(self.core_sharding for _ in out_avals))

        if nc.dbg_addr is not None and nc.dbg_callbacks:
            raise RuntimeError("dbg callbacks unsupported in this runner")
        self.dbg_name = nc.dbg_addr.name if nc.dbg_addr is not None else None

    def put_params(self, shared, pkey):
        """Ship replicated params + per-core ELL indices to the device."""
        jax = self.jax
        self.static_dev = {}
        for name in self.in_names:
            if name == "xT":
                continue
            if name == "idx":
                arr = np.ascontiguousarray(
                    self.sched["idxs"].astype(np.uint16).reshape(
                        NCORES * 128, -1))
            elif name == self.dbg_name:
                arr = np.zeros((NCORES, 2), np.uint32)
            else:
                a = shared[name]
                arr = np.ascontiguousarray(
                    np.broadcast_to(a, (NCORES,) + a.shape).reshape(
                        NCORES * a.shape[0], *a.shape[1:]))
            self.static_dev[name] = jax.device_put(arr, self.core_sharding)
        jax.block_until_ready(list(self.static_dev.values()))
        self.pkey = pkey

    def run(self, xT_bf):
        """xT_bf: (NCORES*F0, NPADL) bfloat16. Fully async dispatch: H2D,
        zero-buffer creation and the SPMD exec are enqueued without
        intermediate blocking; np.asarray at the end syncs once."""
        jax = self.jax
        xT_dev = jax.device_put(xT_bf, self.core_sharding)
        zeros = self.make_zeros()
        args = [xT_dev if n == "xT" else self.static_dev[n]
                for n in self.in_names] + list(zeros)
        outs = self.sharded(*args)
        self._last_xT = xT_dev
        return np.asarray(outs[0])

    def measure_exec_ns(self, K=16, batches=3):
        """Steady-state device execution time per call: enqueue K back-to-back
        executions on device-resident inputs, sync once, divide by K; best of
        `batches`. Amortizes the ~70 ms axon tunnel round-trip latency that a
        single-dispatch measurement cannot separate from device time."""
        jax = self.jax
        assert self._last_xT is not None
        args0 = [self._last_xT if n == "xT" else self.static_dev[n]
                 for n in self.in_names]
        best = float("inf")
        for _ in range(batches):
            try:
                zsets = [self.make_zeros() for _ in range(K)]
                jax.block_until_ready(zsets)
                jax.block_until_ready(args0)
                t0 = time.perf_counter()
                outs = [self.sharded(*args0, *zsets[k]) for k in range(K)]
                jax.block_until_ready(outs)
                best = min(best, (time.perf_counter() - t0) / K)
            except Exception:
                if best < float("inf"):
                    break  # keep what we have
                raise
        return best * 1e9


def measure_exec_ns(K=16, batches=3):
    """Pipelined device-exec timing of the most recently used runner."""
    assert _RUNNERS, "call kernel() first"
    r = _RUNNERS[next(reversed(_RUNNERS))]
    return r.measure_exec_ns(K=K, batches=batches)


# ================================================================ entry point
def kernel(**inputs):
    x = np.ascontiguousarray(np.asarray(inputs["x"], dtype=np.float32))
    edge_index = np.asarray(inputs["edge_index"])
    N, F_in = x.shape
    H = inputs["a_src1"].shape[0]
    C = inputs["a_src1"].shape[1]
    C_out = inputs["a_src3"].shape[1]
    cfgs = _layer_cfgs(F_in, H, C, C_out)

    ekey = (N, F_in, H, C, C_out, TAB_F32, _digest(edge_index))
    runner = _RUNNERS.get(ekey)
    if runner is None:
        sched = _schedule(edge_index, N)
        runner = _Runner(sched, cfgs, ekey)
        _RUNNERS[ekey] = runner

    sched = runner.sched
    Pi = sched["Pi"]
    NLOC, NPADL = sched["NLOC"], sched["NPADL"]

    Ws = [np.asarray(inputs[k], dtype=np.float32) for k in ("W1", "W2", "W3")]
    A = [np.concatenate(
        [np.asarray(inputs[f"a_src{l}"], dtype=np.float32),
         np.asarray(inputs[f"a_dst{l}"], dtype=np.float32)], axis=0)
        for l in (1, 2, 3)]
    Bs = [np.asarray(inputs[k], dtype=np.float32) for k in ("b1", "b2", "b3")]
    shared = {}
    for l in range(3):
        shared[f"w{l}"] = _to_bf16(Ws[l])
        shared[f"a{l}"] = _to_bf16(A[l])
        shared[f"b{l}"] = np.ascontiguousarray(Bs[l][None, :])
    pkey = _digest(*[shared[k] for k in sorted(shared)])
    if runner.pkey != pkey:
        runner.put_params(shared, pkey)

    # ---- per-call: permute + transpose + bf16-round the node features
    u32 = x.view(np.uint32)
    r = ((u32 + np.uint32(0x7FFF) + ((u32 >> np.uint32(16)) & np.uint32(1)))
         >> np.uint32(16)).astype(np.uint16)          # f32 -> bf16 (RNE)
    rt = np.zeros((NCORES, F_in, NPADL), np.uint16)
    rt[:, :, :NLOC] = r[Pi].reshape(NCORES, NLOC, F_in).transpose(0, 2, 1)
    import ml_dtypes
    xT_bf = rt.reshape(NCORES * F_in, NPADL).view(ml_dtypes.bfloat16)

    out_bf = runner.run(xT_bf)

    # ---- un-permute; bf16 -> f32
    full = (out_bf.view(np.uint16).astype(np.uint32) << np.uint32(16)) \
        .view(np.float32).reshape(NCORES, NPADL, C_out)
    out = np.empty((N, C_out), dtype=np.float32)
    for c in range(NCORES):
        out[Pi[c * NLOC:(c + 1) * NLOC]] = full[c, :NLOC]
    return out
